# revision 63
# baseline (speedup 1.0000x reference)
"""Trainium2 Bass kernel for a dense transformer block (pre-LN attention + GELU MLP).

Strategy: data-parallel over batch across 8 NeuronCores (2 batches/core, no
collectives).  Per core: fp8e4 DoubleRow matmuls for the projections (1-term),
PV (1-term), wo (1-term) and both MLP layers (3-term hi+lo error-compensated,
~bf16 accuracy); fp8 scores; bf16 residual stream and transposes.  LN gains
fold into the weights on the host, LN/projection biases fold into per-feature
effective biases.  The two batches are software-pipelined so batch1's
Act-bound attention overlaps batch0's PE-bound MLP.
"""

import numpy as np
import ml_dtypes

import concourse.bass as bass
import concourse.mybir as mybir
import concourse.tile as tile
from concourse import bacc, bass_utils
from concourse.masks import make_identity

# Problem shape (hardcoded per spec nn_Block_58652073394865)
B, S, D, H, F = 16, 577, 1024, 16, 4096
DH = D // H
NCORES = 8
BL = B // NCORES
P = 128
KK = D // P              # 8 chunks of the model dim
FK = F // P              # 32 chunks of the mlp dim
CP = KK // 2             # 4 DoubleRow chunk-pairs for D
CP2 = FK // 2            # 16 DoubleRow chunk-pairs for F
EPS = 1e-6

SP = 592                 # token stride: 577 -> 578 (pad token) -> 592 so
                         # DoubleRow ldweights APs stay 16B-aligned
TT = [(0, 128), (128, 128), (256, 128), (384, 128), (512, 66)]
QC = [(0, 290), (288, 290)]          # moving-token chunks (even, overlap ok)
DC = [(0, 512), (512, 512)]          # model-dim 512 halves
VS = 80                  # per-head stride in v (64 v + 1 ones + 15 pad,
                         # 16B-aligned for DoubleRow ldweights)

SW = 64.0                # weight prescale into e4m3 range
SW2 = 128.0              # w2 prescale
KEXP = 5.0               # softmax exp bias: es = exp(s/8 - KEXP*ln2)
LN2C = 0.6931471805599453

F32 = mybir.dt.float32
BF16 = mybir.dt.bfloat16
FP8 = mybir.dt.float8e4
E4NP = ml_dtypes.float8_e4m3
BFNP = ml_dtypes.bfloat16
AF = mybir.ActivationFunctionType
OP = mybir.AluOpType
DR = mybir.MatmulPerfMode.DoubleRow

_NC_CACHE = None
# CoreSim doesn't implement the Gelu LUT; tests may swap this for AF.Tanh
_GELU = AF.Gelu


def _build():
    nc = bacc.Bacc("TRN2", target_bir_lowering=False, debug=False,
                   num_devices=NCORES)

    x_d = nc.dram_tensor("x", [BL, S, D], BF16, kind="ExternalInput").ap()
    y_d = nc.dram_tensor("y", [BL, S, D], F32, kind="ExternalOutput").ap()
    # packed fp8 weights, host-permuted to [p, chunkpair, 2, m] layout
    wq_d = nc.dram_tensor("wq8", [P, CP, 2, D], FP8, kind="ExternalInput").ap()
    wk_d = nc.dram_tensor("wk8", [P, CP, 2, D], FP8, kind="ExternalInput").ap()
    wv_d = nc.dram_tensor("wv8", [P, CP, 2, D], FP8, kind="ExternalInput").ap()
    wo_d = nc.dram_tensor("wo8", [P, CP, 2, D], FP8, kind="ExternalInput").ap()
    w1_d = nc.dram_tensor("w1p", [P, CP, 2, 2, F], FP8, kind="ExternalInput").ap()
    w2_d = nc.dram_tensor("w2p", [KK, P, CP2, 2, 2, P], FP8, kind="ExternalInput").ap()
    bo_d = nc.dram_tensor("bo8", [D], FP8, kind="ExternalInput").ap()
    cap_d = nc.dram_tensor("cap", [P, 3 * KK + FK], F32, kind="ExternalInput").ap()

    with tile.TileContext(nc) as tc:
        with tc.tile_pool(name="const", bufs=1) as cpool, \
             tc.tile_pool(name="resid", bufs=4) as rpool, \
             tc.tile_pool(name="fm", bufs=2) as fmpool, \
             tc.tile_pool(name="h1p", bufs=1) as h1pool, \
             tc.tile_pool(name="small", bufs=2) as spool, \
             tc.tile_pool(name="lnp", bufs=2) as lnpool, \
             tc.tile_pool(name="psa", bufs=2, space="PSUM") as psA, \
             tc.tile_pool(name="pss", bufs=2, space="PSUM") as psS, \
             tc.tile_pool(name="psc", bufs=2, space="PSUM") as psC:

            # ---- constants / small params ----
            cA = cpool.tile([P, 3 * KK + FK], F32, tag="cA")
            bq_sb = cA[:, 0:KK]
            bk_sb = cA[:, KK:2 * KK]
            b2_sb = cA[:, 2 * KK:3 * KK]
            b1_sb = cA[:, 3 * KK:3 * KK + FK]
            def load_small_params():
                # one packed DMA for all small params; Pool queue is idle
                # until attention so the holds are free
                nc.gpsimd.dma_start(cA[:], cap_d)
                nc.gpsimd.dma_start(t_bo8, bo_d[None, :])

            cB = cpool.tile([P, P + 3], F32, tag="cB")
            identf = cB[:, 0:P]
            epsap = cB[:, P:P + 1]
            expb = cB[:, P + 1:P + 2]       # softmax exp bias -KEXP*ln2
            scratch1 = cB[:, P + 2:P + 3]
            make_identity(nc, identf)
            nc.vector.memset(epsap, EPS)
            nc.vector.memset(expb, -KEXP * LN2C)
            # dummy exp pre-triggers the act-table load off the critical path
            nc.scalar.activation(scratch1[:1], epsap[:1], AF.Exp)
            ident = cpool.tile([P, P], BF16, tag="identb")
            nc.vector.tensor_copy(ident[:], identf)

            c8 = cpool.tile([1, P + D], FP8, tag="c8")
            ones8 = c8[:, 0:P]
            t_bo8 = c8[:, P:P + D]
            nc.vector.memset(ones8, 1.0)

            onec = cpool.tile([P, 1], FP8, tag="onec")
            nc.vector.memset(onec[:], 1.0)

            # ---- per-batch big tiles (tag-rotated across batches) ----
            def batch_tiles(b):
                t = {}
                t["xb"] = rpool.tile([P, 5, D], BF16, tag="resid", name=f"xb{b}")
                t["xn8"] = fmpool.tile([P, KK, SP], FP8, tag="xn8", name=f"xn8_{b}")
                t["q8"] = fmpool.tile([P, KK, SP], FP8, tag="q8", name=f"q8_{b}")
                t["k8"] = fmpool.tile([P, KK, SP], FP8, tag="k8", name=f"k8_{b}")
                t["v8"] = fmpool.tile([P, 5, H * VS], FP8, tag="v8", name=f"v8_{b}")
                t["ctx8"] = fmpool.tile([P, KK, SP], FP8, tag="ctx8", name=f"ctx8_{b}")
                return t

            # ---- layernorm helpers ----
            # stats cols: 0:5 negmu, 5:10 sumsq, 10:15 mu2, 15:20 var,
            # 20:25 lnv, 25:30 rsig
            def ln_new_stats():
                st = lnpool.tile([P, 35], F32, tag="stats", name="stats")
                nc.vector.memset(st[:, 0:5], 0.0)
                nc.vector.memset(st[:, 5:10], float(D))
                return st

            def ln_tile_stats(st, src, ti, pt, act_only=False):
                # sum-of-x on Act (act_only, keeps DVE clear at startup) or
                # DVE (tensor_reduce); sum-of-squares always on Act
                scr = lnpool.tile([P, D], BF16, tag="scr", bufs=1)
                if act_only:
                    nc.scalar.activation(scr[:pt], src[:pt, ti], AF.Identity,
                                         accum_out=st[:pt, ti:ti + 1])
                else:
                    nc.vector.tensor_reduce(st[:pt, ti:ti + 1], src[:pt, ti],
                                            mybir.AxisListType.X, OP.add)
                nc.scalar.activation(scr[:pt], src[:pt, ti], AF.Square,
                                     accum_out=st[:pt, 5 + ti:6 + ti])

            def ln_finalize(st, lo, hi):
                # rsig = rsqrt(var+eps) = exp(-0.5*ln(v)) with ln(v) by Taylor
                # around v=1 (v is ~1 +- 0.3 here), sharpened by one Newton
                # step.  Exp shares the softmax act table, so no table swaps.
                sumx = st[:, 0 + lo:0 + hi]
                negmu = st[:, 0 + lo:0 + hi]
                ssq = st[:, 5 + lo:5 + hi]
                t1 = st[:, 10 + lo:10 + hi]
                var = st[:, 15 + lo:15 + hi]
                u = st[:, 20 + lo:20 + hi]
                rsig = st[:, 25 + lo:25 + hi]
                p = st[:, 30 + lo:30 + hi]
                nc.vector.tensor_scalar_mul(negmu, sumx, -1.0 / D)
                nc.vector.tensor_tensor(t1, negmu, negmu, OP.mult)
                nc.vector.scalar_tensor_tensor(var, ssq, 1.0 / D, t1,
                                               OP.mult, OP.subtract)
                nc.vector.tensor_scalar(var, var, 1.0, EPS, OP.mult, OP.add)
                nc.vector.tensor_scalar(u, var, 1.0, -1.0, OP.mult, OP.add)
                nc.vector.tensor_scalar(p, u, 0.25, -0.5, OP.mult, OP.add)
                nc.vector.tensor_tensor(u, u, p, OP.mult)
                nc.scalar.activation(rsig, u, AF.Exp)
                nc.vector.tensor_tensor(t1, rsig, rsig, OP.mult)
                nc.vector.tensor_tensor(t1, t1, var, OP.mult)
                nc.vector.tensor_scalar(t1, t1, -0.5, 1.5, OP.mult, OP.add)
                nc.vector.tensor_tensor(rsig, rsig, t1, OP.mult)

            def ln_apply(st, src, ti, pt, outs):
                # z = (x - mu) * rsig  (bf16, token-major), then transpose to
                # feature-major and write fp8 via `outs(pgrp, kk0, nkk, t0, pt)`
                t0 = TT[ti][0]
                z = lnpool.tile([P, D], BF16, tag="z")
                nc.vector.tensor_scalar(z[:pt], src[:pt, ti],
                                        st[:pt, ti:ti + 1],
                                        st[:pt, 25 + ti:26 + ti],
                                        OP.add, OP.mult)
                for g in range(2):
                    pT = psC.tile([P, 512], BF16, tag="pC")
                    for j in range(4):
                        kk = g * 4 + j
                        nc.tensor.transpose(pT[:, j * P:j * P + pt],
                                            z[:pt, kk * P:(kk + 1) * P],
                                            ident[:pt, :pt])
                    pgrp = pT[:].rearrange("p (j c) -> p j c", j=4)[:, :, :pt]
                    outs(pgrp, g * 4, 4, t0, pt)

            # ==== stage emitters (generators yield emission quanta) ====

            def ln1_quanta(b, t):
                def q0():
                    nc.vector.memset(t["xb"][64:, 4, :], 0.0)
                    # tile 0 alone first (its LN chain is the startup
                    # critical path), then tiles 1-3 as one transfer
                    nc.sync.dma_start(t["xb"][:, 0, :], x_d[b, 0:P, :])
                    nc.sync.dma_start(
                        t["xb"][:, 1:4, :],
                        x_d[b, P:512].rearrange("(t p) d -> p t d", p=P))
                    nc.sync.dma_start(t["xb"][:65, 4], x_d[b, 512:S, :])
                    t["st1"] = ln_new_stats()
                yield q0

                def xn_out(pgrp, kk0, nkk, t0, pt):
                    nc.vector.tensor_copy(
                        t["xn8"][:, kk0:kk0 + nkk, t0:t0 + pt], pgrp)

                def fin0():
                    ln_tile_stats(t["st1"], t["xb"], 0, 128, act_only=(b == 0))
                    ln_finalize(t["st1"], 0, 1)
                    ln_apply(t["st1"], t["xb"], 0, 128, xn_out)
                yield fin0

                def fin1():
                    for ti in (1, 2, 3):
                        ln_tile_stats(t["st1"], t["xb"], ti, 128,
                                      act_only=(b == 0))
                    ln_finalize(t["st1"], 1, 4)
                yield fin1
                for ti in (1, 2, 3):
                    yield lambda ti=ti: ln_apply(t["st1"], t["xb"], ti, TT[ti][1], xn_out)

                def fin4():
                    ln_tile_stats(t["st1"], t["xb"], 4, 66, act_only=(b == 0))
                    ln_finalize(t["st1"], 4, 5)
                    ln_apply(t["st1"], t["xb"], 4, 66, xn_out)
                yield fin4

            def proj_quanta(b, t, wq_sb, wk_sb, wv_sb):
                # q/k per (m-tile, token-chunk); QC0 chunks first since they
                # only need xn8 token tiles 0-2 (tile 4 lands last at startup)
                def qk_m(m, qi):
                    q0, qn = QC[qi]
                    for w_sb, dst, bias in ((wq_sb, t["q8"], bq_sb),
                                            (wk_sb, t["k8"], bk_sb)):
                        ps = psA.tile([P, 512], F32, tag="pA")
                        for c in range(CP):
                            nc.tensor.matmul(
                                ps[:, :qn],
                                w_sb[:, c, :, m * P:(m + 1) * P],
                                t["xn8"][:, 2 * c:2 * c + 2, q0:q0 + qn],
                                start=(c == 0), stop=(c == CP - 1),
                                perf_mode=DR)
                        nc.vector.tensor_scalar(
                            dst[:, m, q0:q0 + qn], ps[:, :qn],
                            1.0 / SW, bias[:, m:m + 1], OP.mult, OP.add)
                for m in range(KK):
                    yield lambda m=m: qk_m(m, 0)
                for m in range(KK):
                    yield lambda m=m: qk_m(m, 1)
                # v: ones column / pad-row zeroing, then the projection
                def v_init():
                    v_hc = t["v8"][:].rearrange("p t (h c) -> p t h c", c=VS)
                    nc.vector.memset(v_hc[64:, 4:5], 0.0)
                    nc.vector.memset(v_hc[:, :, :, 65:VS], 0.0)
                    nc.vector.tensor_copy(
                        v_hc[:, 0:4, :, 64:65],
                        onec[:, :, None, None].to_broadcast((P, 4, H, 1)))
                    nc.vector.tensor_copy(
                        v_hc[:65, 4:5, :, 64:65],
                        onec[:65, :, None, None].to_broadcast((65, 1, H, 1)))
                yield v_init
                for ci, (c0, cn) in enumerate(DC):
                    for ti, (t0, pt) in enumerate(TT):
                        def v_t(ci=ci, c0=c0, cn=cn, ti=ti, t0=t0, pt=pt):
                            ps = psA.tile([P, 512], F32, tag="pA")
                            for c in range(CP):
                                nc.tensor.matmul(
                                    ps[:pt],
                                    t["xn8"][:, 2 * c:2 * c + 2, t0:t0 + pt],
                                    wv_sb[:, c, :, c0:c0 + cn],
                                    start=(c == 0), stop=(c == CP - 1),
                                    perf_mode=DR)
                            rp = min(pt, S - t0)
                            nc.vector.tensor_scalar_mul(
                                t["v8"][:rp, ti].rearrange(
                                    "p (h c) -> p h c", c=VS)[:, ci * 8:(ci + 1) * 8, 0:64],
                                ps[:rp, :cn].rearrange("p (h c) -> p h c", c=64),
                                1.0 / SW)
                        yield v_t

            def attn_quanta(b, t):
                # per (head, qc) chunk: scores + exp; pv/normalize for the
                # previous chunk trails one step so exp latency is hidden
                def scores_exp(h, qi):
                    hrow = (h % 2) * 64
                    kkh = h // 2
                    q0, qn = QC[qi]
                    es = spool.tile([P, 5, 290], FP8, tag=f"es{qi}", bufs=2)
                    t[f"es_{h}_{qi}"] = es
                    # token tiles 0-3 share one 4-bank psum group -> one Exp
                    pg = psS.tile([P, 4, 512], F32, tag="pS", bufs=1)
                    for kt in range(4):
                        kt0, ptk = TT[kt]
                        nc.tensor.matmul(
                            pg[:ptk, kt, :qn],
                            t["k8"][hrow:hrow + 64, kkh, kt0:kt0 + ptk],
                            t["q8"][hrow:hrow + 64, kkh, q0:q0 + qn],
                            start=True, stop=True)
                    nc.scalar.activation(
                        es[:, 0:4, :qn], pg[:, :, :qn],
                        AF.Exp, scale=1.0 / np.sqrt(DH), bias=expb[:])
                    pg4 = psA.tile([P, 512], F32, tag="pA")
                    kt0, ptk = TT[4]
                    nc.tensor.matmul(
                        pg4[:ptk, :qn],
                        t["k8"][hrow:hrow + 64, kkh, kt0:kt0 + ptk],
                        t["q8"][hrow:hrow + 64, kkh, q0:q0 + qn],
                        start=True, stop=True)
                    nc.scalar.activation(
                        es[:ptk, 4, :qn], pg4[:ptk, :qn],
                        AF.Exp, scale=1.0 / np.sqrt(DH), bias=expb[:ptk])

                def pv_norm(h, qi):
                    hrow = (h % 2) * 64
                    kkh = h // 2
                    q0, qn = QC[qi]
                    es = t.pop(f"es_{h}_{qi}")
                    pc = psC.tile([P, 512], F32, tag="pC")
                    for kt in (0, 2):
                        nc.tensor.matmul(
                            pc[:VS, :qn],
                            t["v8"][:, kt:kt + 2, h * VS:(h + 1) * VS],
                            es[:, kt:kt + 2, :qn],
                            start=(kt == 0), stop=False, perf_mode=DR)
                    nc.tensor.matmul(
                        pc[:VS, :qn], t["v8"][:66, 4, h * VS:(h + 1) * VS],
                        es[:66, 4, :qn], start=False, stop=True)
                    rc = spool.tile([1, 290], F32, tag="rc", bufs=2)
                    nc.vector.reciprocal(rc[:, :qn], pc[64:65, :qn])
                    rb = spool.tile([64, 290], F32, tag="rb", bufs=2)
                    nc.gpsimd.partition_broadcast(rb[:, :qn], rc[:, :qn])
                    # ctx stays on DVE (reads PSUM); move the copy of pc out
                    # of PSUM... (gpsimd is SBUF-only, so DVE does this one)
                    nc.vector.tensor_tensor(
                        t["ctx8"][hrow:hrow + 64, kkh, q0:q0 + qn],
                        pc[0:64, :qn], rb[:, :qn], OP.mult)

                prev = None
                for h in range(H):
                    for qi in range(2):
                        def quantum(h=h, qi=qi, prev=prev):
                            scores_exp(h, qi)
                            if prev is not None:
                                pv_norm(*prev)
                        yield quantum
                        prev = (h, qi)
                yield lambda: pv_norm(*prev)

            def stage_e_quanta(b, t, wstream):
                # x2 = ctx@wo/SW + bo + xb, with LN2 stats folded per tile;
                # wo streamed per 512-column half to save SBUF
                def q0():
                    t["x2"] = rpool.tile([P, 5, D], BF16, tag="resid",
                                         name=f"x2_{b}")
                    t["st2"] = ln_new_stats()
                yield q0
                for ci, (c0, cn) in enumerate(DC):
                    def ldwo(ci=ci, c0=c0, cn=cn):
                        wo_sb = wstream.tile([P, CP, 2, 512], FP8, tag="wo",
                                             bufs=1, name=f"wo{ci}_{b}")
                        t["wo_sb"] = wo_sb
                        nc.sync.dma_start(wo_sb[:], wo_d[:, :, :, c0:c0 + cn])
                    yield ldwo
                    for ti, (t0, pt) in enumerate(TT):
                        def eq(ci=ci, c0=c0, cn=cn, ti=ti, t0=t0, pt=pt):
                            wo_sb = t["wo_sb"]
                            ps = psA.tile([P, 512], F32, tag="pA")
                            nc.tensor.matmul(ps[:pt, :cn], ones8[:, :pt],
                                             t_bo8[:, c0:c0 + cn],
                                             start=True, stop=False)
                            for c in range(CP):
                                nc.tensor.matmul(
                                    ps[:pt, :cn],
                                    t["ctx8"][:, 2 * c:2 * c + 2, t0:t0 + pt],
                                    wo_sb[:, c, :, :cn],
                                    start=False, stop=(c == CP - 1),
                                    perf_mode=DR)
                            nc.vector.scalar_tensor_tensor(
                                t["x2"][:pt, ti, c0:c0 + cn], ps[:pt, :cn],
                                1.0 / SW, t["xb"][:pt, ti, c0:c0 + cn],
                                OP.mult, OP.add)
                            if ci == len(DC) - 1:
                                ln_tile_stats(t["st2"], t["x2"], ti, pt)
                        yield eq

            def ln2_quanta(b, t):
                def q0():
                    t["x2h"] = fmpool.tile([P, KK, SP], FP8, tag="x2h",
                                           bufs=1, name=f"x2h_{b}")
                    t["x2l"] = fmpool.tile([P, KK, SP], FP8, tag="x2l",
                                           bufs=1, name=f"x2l_{b}")
                    ln_finalize(t["st2"], 0, 1)
                yield q0

                def xn2_out(pgrp, kk0, nkk, t0, pt):
                    # hi-copy on Act (idle here), lo on DVE: halves the
                    # serial DVE chain that gates fc1
                    hi = t["x2h"][:, kk0:kk0 + nkk, t0:t0 + pt]
                    nc.scalar.copy(hi, pgrp)
                    nc.vector.scalar_tensor_tensor(
                        t["x2l"][:, kk0:kk0 + nkk, t0:t0 + pt],
                        pgrp, 1.0, hi, OP.mult, OP.subtract)

                for ti, (t0, pt) in enumerate(TT):
                    def ap(ti=ti, t0=t0, pt=pt):
                        if ti == 1:
                            ln_finalize(t["st2"], 1, 5)
                        ln_apply(t["st2"], t["x2"], ti, pt, xn2_out)
                    yield ap

            def fc1_quanta(b, t, wstream):
                # fc1: 3-term DR; gelu once to bf16 tmp, split hi/lo from it.
                # Kept in its own phase so Gelu never interleaves with Exp
                # (each act-table swap costs 1.3us on the Act engine).
                h1h = h1pool.tile([P, FK, SP], FP8, tag="h1h", name=f"h1h_{b}")
                h1l = h1pool.tile([P, FK, SP], FP8, tag="h1l", name=f"h1l_{b}")
                t["h1h"], t["h1l"] = h1h, h1l

                def ldw1(blk):
                    wb = wstream.tile([P, CP, 2, 2, 512], FP8, tag="w1",
                                      bufs=2, name=f"w1b{blk}")
                    t[f"w1_{blk}"] = wb
                    nc.sync.dma_start(
                        wb[:], w1_d[:, :, :, :, blk * 512:(blk + 1) * 512])

                for blk in range(8):          # 512-feature blocks
                    # prefetch distance 1: load blk+1 while blk computes
                    if blk == 0:
                        yield lambda: (ldw1(0), ldw1(1))
                    elif blk < 7:
                        yield lambda blk=blk: ldw1(blk + 1)
                    for mi in range(4):
                        def fc1_m(blk=blk, mi=mi):
                            wb = t[f"w1_{blk}"]
                            m = blk * 4 + mi
                            htmp = spool.tile([P, SP], BF16, tag="htmp", bufs=2)
                            terms = ((0, t["x2h"]), (1, t["x2h"]), (0, t["x2l"]))
                            for (q0, qn) in QC:
                                ps = psA.tile([P, 512], F32, tag="pA")
                                n = CP * len(terms)
                                i = 0
                                for c in range(CP):
                                    for wj, xh in terms:
                                        nc.tensor.matmul(
                                            ps[:, :qn],
                                            wb[:, c, :, wj, mi * P:(mi + 1) * P],
                                            xh[:, 2 * c:2 * c + 2, q0:q0 + qn],
                                            start=(i == 0), stop=(i == n - 1),
                                            perf_mode=DR)
                                        i += 1
                                nc.scalar.activation(
                                    htmp[:, q0:q0 + qn], ps[:, :qn], _GELU,
                                    scale=1.0 / SW, bias=b1_sb[:, m:m + 1])
                            nc.vector.tensor_copy(h1h[:, m, :], htmp[:])
                            nc.gpsimd.tensor_tensor(
                                h1l[:, m, :], htmp[:], h1h[:, m, :], OP.subtract)
                        yield fc1_m

            def fc2_quanta(b, t, wstream):
                # fc2: 3-term DR over F; readout -> bf16 -> transpose -> +x2
                h1h, h1l = t["h1h"], t["h1l"]
                for m in range(KK):
                    def ldw2(m=m):
                        wb = wstream.tile([P, CP2, 2, 2, P], FP8, tag="w2",
                                          bufs=2, name=f"w2b{m}")
                        t[f"w2_{m}"] = wb
                        nc.gpsimd.dma_start(wb[:], w2_d[m])
                    yield ldw2

                    def fc2_m(m=m):
                        wb = t.pop(f"w2_{m}")
                        mlp_m = spool.tile([P, SP], BF16, tag="mlpm", bufs=2)
                        terms = ((0, h1h), (1, h1h), (0, h1l))
                        for (q0, qn) in QC:
                            ps = psA.tile([P, 512], F32, tag="pA")
                            n = CP2 * len(terms)
                            i = 0
                            for c in range(CP2):
                                for wj, xh in terms:
                                    nc.tensor.matmul(
                                        ps[:, :qn],
                                        wb[:, c, :, wj, :],
                                        xh[:, 2 * c:2 * c + 2, q0:q0 + qn],
                                        start=(i == 0), stop=(i == n - 1),
                                        perf_mode=DR)
                                    i += 1
                            nc.vector.tensor_scalar(
                                mlp_m[:, q0:q0 + qn], ps[:, :qn],
                                1.0 / SW2, b2_sb[:, m:m + 1], OP.mult, OP.add)
                        # transpose back token-major, add residual, stage for DMA
                        pT = psC.tile([P, 512], BF16, tag="pC")
                        for ti in range(4):
                            nc.tensor.transpose(
                                pT[:, ti * P:(ti + 1) * P],
                                mlp_m[:, TT[ti][0]:TT[ti][0] + 128],
                                ident[:])
                        pT2 = psC.tile([P, 512], BF16, tag="pC")
                        nc.tensor.transpose(pT2[:66, :P], mlp_m[:, 512:578],
                                            ident[:])
                        if m % 4 == 0:
                            t["og"] = [spool.tile([P, 4, P], F32, tag="og", bufs=5,
                                                  name=f"og{ti}")
                                       for ti in range(5)]
                        for ti, (t0, pt) in enumerate(TT):
                            src = (pT[:pt, ti * P:(ti + 1) * P] if ti < 4
                                   else pT2[:pt, 0:P])
                            nc.vector.scalar_tensor_tensor(
                                t["og"][ti][:pt, m % 4, :], src, 1.0,
                                t["x2"][:pt, ti, m * P:(m + 1) * P],
                                OP.mult, OP.add)
                        if m % 4 == 3:
                            mg = m // 4
                            for ti, (t0, pt) in enumerate(TT):
                                rp = min(pt, S - t0)
                                eng = nc.gpsimd if ti % 2 == 0 else nc.sync
                                eng.dma_start(
                                    y_d[b, t0:t0 + rp, mg * 512:(mg + 1) * 512],
                                    t["og"][ti][:rp])
                    yield fc2_m

            # ==== driver: software-pipeline the two batches ====

            def run_seq(gens):
                for g in gens:
                    for q in g:
                        q()

            def interleave(ga, gb, ratio=1):
                # ratio quanta of ga per quantum of gb
                ga, gb = iter(ga), iter(gb)
                alive_a = alive_b = True
                while alive_a or alive_b:
                    for _ in range(ratio):
                        if alive_a:
                            try:
                                next(ga)()
                            except StopIteration:
                                alive_a = False
                    if alive_b:
                        try:
                            next(gb)()
                        except StopIteration:
                            alive_b = False

            def chain(*gens):
                for g in gens:
                    yield from g

            tiles = [batch_tiles(0), batch_tiles(1)]

            with tc.tile_pool(name="wqkv", bufs=1) as qkvpool:
                wq_sb = qkvpool.tile([P, CP, 2, D], FP8, tag="wq")
                wk_sb = qkvpool.tile([P, CP, 2, D], FP8, tag="wk")
                wv_sb = qkvpool.tile([P, CP, 2, D], FP8, tag="wv")
                # x(0) first so its tiles win the serial DMA device, then
                # weights on the Pool queue (idle until attention)
                g_ln1 = iter(ln1_quanta(0, tiles[0]))
                next(g_ln1)()
                nc.gpsimd.dma_start(wq_sb[:], wq_d)
                nc.gpsimd.dma_start(wk_sb[:], wk_d)
                nc.gpsimd.dma_start(wv_sb[:], wv_d)
                load_small_params()

                # ph0: LN1(0) + proj(0)
                run_seq([g_ln1,
                         proj_quanta(0, tiles[0], wq_sb, wk_sb, wv_sb)])
                # ph1: attention(0) [Act-bound] vs LN1(1)+proj(1) [PE-bound]
                interleave(attn_quanta(0, tiles[0]),
                           chain(ln1_quanta(1, tiles[1]),
                                 proj_quanta(1, tiles[1], wq_sb, wk_sb, wv_sb)))

            def take(g, n):
                for _ in range(n):
                    try:
                        yield next(g)
                    except StopIteration:
                        return

            with tc.tile_pool(name="wstr", bufs=1) as wstream:
                ag1 = iter(attn_quanta(1, tiles[1]))
                # ph2: stage E(0)+LN2(0) [serial-chain heavy] filled with the
                # first attention(1) chunks (same act table as LN)
                interleave(chain(stage_e_quanta(0, tiles[0], wstream),
                                 ln2_quanta(0, tiles[0])),
                           take(ag1, 12))
                # ph2.5: fc1(0) alone (PE-bound; keeps Gelu table resident)
                run_seq([fc1_quanta(0, tiles[0], wstream)])
                # ph3: fc2(0) [PE-bound, no Act] vs attention(1) [Act-bound];
                # hold back the last 2 fc2 m-tiles to fill ph4's chains
                fg0 = iter(fc2_quanta(0, tiles[0], wstream))
                interleave(ag1, take(fg0, 12), ratio=2)
                # ph4: stage E(1) + LN2(1), filled with the fc2(0) tail
                fg1 = iter(fc1_quanta(1, tiles[1], wstream))
                next(fg1)()      # prefetch w1(1) blocks 0-1 during ph4
                interleave(chain(stage_e_quanta(1, tiles[1], wstream),
                                 ln2_quanta(1, tiles[1])),
                           fg0, ratio=4)
                # ph4.5 + ph5: MLP(1); first w1 blocks prefetched in ph4
                run_seq([fg1, fc2_quanta(1, tiles[1], wstream)])

    nc.compile()
    return nc


def _get_nc():
    global _NC_CACHE
    if _NC_CACHE is None:
        _NC_CACHE = _build()
    return _NC_CACHE


def _prep_inputs(inputs):
    """Host-side weight packing: LN-gain folds, bias folds, fp8 hi/lo splits."""
    f = {k: np.asarray(v, np.float32) for k, v in inputs.items()}
    g1, b1g = f["ln1_g"], f["ln1_b"]
    g2, b2g = f["ln2_g"], f["ln2_b"]

    def pack_d(w, ncp=CP):
        # [K, M] -> [P, ncp, 2, M] with K = ncp*2*P
        return np.ascontiguousarray(
            w.reshape(ncp, 2, P, -1).transpose(2, 0, 1, 3))

    def q8(a):
        return a.astype(E4NP)

    def split(a):
        hi = a.astype(E4NP)
        lo = (a - hi.astype(np.float32)).astype(E4NP)
        return hi, lo

    out = {}
    out["wq8"] = q8(pack_d(g1[:, None] * f["wq"] * SW))
    out["wk8"] = q8(pack_d(g1[:, None] * f["wk"] * SW))
    out["wv8"] = q8(pack_d(g1[:, None] * f["wv"] * SW))
    out["wo8"] = q8(pack_d(f["wo"] * SW))
    w1h, w1l = split(pack_d(g2[:, None] * f["w1"] * SW))
    out["w1p"] = np.ascontiguousarray(np.stack([w1h, w1l], axis=3))
    w2s = f["w2"] * SW2
    w2m = np.stack([pack_d(w2s[:, m * P:(m + 1) * P], CP2) for m in range(KK)])
    w2h, w2l = split(w2m)
    out["w2p"] = np.ascontiguousarray(np.stack([w2h, w2l], axis=4))

    bq_eff = b1g @ f["wq"] + f["bq"]
    bk_eff = b1g @ f["wk"] + f["bk"]
    bv_eff = b1g @ f["wv"] + f["bv"]
    out["bo8"] = ((bv_eff @ f["wo"] + f["bo"]) * SW).astype(E4NP)
    b1_eff = b2g @ f["w1"] + f["b1"]
    cap = np.zeros((P, 3 * KK + FK), np.float32)
    cap[:, 0:KK] = bq_eff.reshape(KK, P).T
    cap[:, KK:2 * KK] = bk_eff.reshape(KK, P).T
    cap[:, 2 * KK:3 * KK] = f["b2"].reshape(KK, P).T
    cap[:, 3 * KK:] = b1_eff.reshape(FK, P).T
    out["cap"] = cap
    return out


def kernel(**inputs):
    nc = _get_nc()
    x = np.asarray(inputs["x"], np.float32).astype(BFNP)
    shared = _prep_inputs(inputs)
    in_maps = []
    for i in range(NCORES):
        m = dict(shared)
        m["x"] = np.ascontiguousarray(x[i * BL:(i + 1) * BL])
        in_maps.append(m)
    res = bass_utils.run_bass_kernel_spmd(nc, in_maps, core_ids=list(range(NCORES)))
    y = np.concatenate([res.results[i]["y"] for i in range(NCORES)], axis=0)
    return y.astype(np.float32)


# revision 66
# speedup vs baseline: 1.0618x; 1.0618x over previous
"""Trainium2 Bass kernel for a dense transformer block (pre-LN attention + GELU MLP).

Strategy: data-parallel over batch across 8 NeuronCores (2 batches/core, no
collectives).  Per core: fp8e4 DoubleRow matmuls for the projections (1-term),
PV (1-term), wo (1-term) and both MLP layers (3-term hi+lo error-compensated,
~bf16 accuracy); fp8 scores; bf16 residual stream and transposes.  LN gains
fold into the weights on the host, LN/projection biases fold into per-feature
effective biases.  The two batches are software-pipelined so batch1's
Act-bound attention overlaps batch0's PE-bound MLP.
"""

import numpy as np
import ml_dtypes

import concourse.bass as bass
import concourse.mybir as mybir
import concourse.tile as tile
from concourse import bacc, bass_utils
from concourse.masks import make_identity

# Problem shape (hardcoded per spec nn_Block_58652073394865)
B, S, D, H, F = 16, 577, 1024, 16, 4096
DH = D // H
NCORES = 8
BL = B // NCORES
P = 128
KK = D // P              # 8 chunks of the model dim
FK = F // P              # 32 chunks of the mlp dim
CP = KK // 2             # 4 DoubleRow chunk-pairs for D
CP2 = FK // 2            # 16 DoubleRow chunk-pairs for F
EPS = 1e-6

SP = 592                 # token stride: 577 -> 578 (pad token) -> 592 so
                         # DoubleRow ldweights APs stay 16B-aligned
TT = [(0, 128), (128, 128), (256, 128), (384, 128), (512, 66)]
QC = [(0, 290), (288, 290)]          # moving-token chunks (even, overlap ok)
DC = [(0, 512), (512, 512)]          # model-dim 512 halves
VS = 80                  # per-head stride in v (64 v + 1 ones + 15 pad,
                         # 16B-aligned for DoubleRow ldweights)

SW = 64.0                # weight prescale into e4m3 range
SW2 = 128.0              # w2 prescale
KEXP = 5.0               # softmax exp bias: es = exp(s/8 - KEXP*ln2)
LN2C = 0.6931471805599453

F32 = mybir.dt.float32
BF16 = mybir.dt.bfloat16
FP8 = mybir.dt.float8e4
E4NP = ml_dtypes.float8_e4m3
BFNP = ml_dtypes.bfloat16
AF = mybir.ActivationFunctionType
OP = mybir.AluOpType
DR = mybir.MatmulPerfMode.DoubleRow

_NC_CACHE = None
# CoreSim doesn't implement the Gelu LUT; tests may swap this for AF.Tanh
_GELU = AF.Gelu


def _build():
    nc = bacc.Bacc("TRN2", target_bir_lowering=False, debug=False,
                   num_devices=NCORES)

    x_d = nc.dram_tensor("x", [BL, S, D], BF16, kind="ExternalInput").ap()
    y_d = nc.dram_tensor("y", [BL, S, D], F32, kind="ExternalOutput").ap()
    # packed fp8 weights, host-permuted to [p, chunkpair, 2, m] layout
    wq_d = nc.dram_tensor("wq8", [P, CP, 2, D], FP8, kind="ExternalInput").ap()
    wk_d = nc.dram_tensor("wk8", [P, CP, 2, D], FP8, kind="ExternalInput").ap()
    wv_d = nc.dram_tensor("wv8", [P, CP, 2, D], FP8, kind="ExternalInput").ap()
    wo_d = nc.dram_tensor("wo8", [P, CP, 2, D], FP8, kind="ExternalInput").ap()
    w1_d = nc.dram_tensor("w1p", [P, CP, 2, 2, F], FP8, kind="ExternalInput").ap()
    w2_d = nc.dram_tensor("w2p", [KK, P, CP2, 2, 2, P], FP8, kind="ExternalInput").ap()
    bo_d = nc.dram_tensor("bo8", [D], FP8, kind="ExternalInput").ap()
    cap_d = nc.dram_tensor("cap", [P, 3 * KK + FK], F32, kind="ExternalInput").ap()

    with tile.TileContext(nc) as tc:
        with tc.tile_pool(name="const", bufs=1) as cpool, \
             tc.tile_pool(name="resid", bufs=4) as rpool, \
             tc.tile_pool(name="fm", bufs=2) as fmpool, \
             tc.tile_pool(name="h1p", bufs=1) as h1pool, \
             tc.tile_pool(name="small", bufs=2) as spool, \
             tc.tile_pool(name="lnp", bufs=2) as lnpool, \
             tc.tile_pool(name="psa", bufs=2, space="PSUM") as psA, \
             tc.tile_pool(name="pss", bufs=2, space="PSUM") as psS, \
             tc.tile_pool(name="psc", bufs=2, space="PSUM") as psC:

            # ---- constants / small params ----
            cA = cpool.tile([P, 3 * KK + FK], F32, tag="cA")
            bq_sb = cA[:, 0:KK]
            bk_sb = cA[:, KK:2 * KK]
            b2_sb = cA[:, 2 * KK:3 * KK]
            b1_sb = cA[:, 3 * KK:3 * KK + FK]
            def load_small_params():
                # one packed DMA for all small params; Pool queue is idle
                # until attention so the holds are free
                nc.gpsimd.dma_start(cA[:], cap_d)
                nc.gpsimd.dma_start(t_bo8, bo_d[None, :])

            cB = cpool.tile([P, P + 3], F32, tag="cB")
            identf = cB[:, 0:P]
            epsap = cB[:, P:P + 1]
            expb = cB[:, P + 1:P + 2]       # softmax exp bias -KEXP*ln2
            scratch1 = cB[:, P + 2:P + 3]
            make_identity(nc, identf)
            nc.vector.memset(epsap, EPS)
            nc.vector.memset(expb, -KEXP * LN2C)
            # dummy exp pre-triggers the act-table load off the critical path
            nc.scalar.activation(scratch1[:1], epsap[:1], AF.Exp)
            ident = cpool.tile([P, P], BF16, tag="identb")
            nc.vector.tensor_copy(ident[:], identf)

            c8 = cpool.tile([1, P + D], FP8, tag="c8")
            ones8 = c8[:, 0:P]
            t_bo8 = c8[:, P:P + D]
            nc.vector.memset(ones8, 1.0)

            onec = cpool.tile([P, 1], FP8, tag="onec")
            nc.vector.memset(onec[:], 1.0)

            # ---- per-batch big tiles (tag-rotated across batches) ----
            def batch_tiles(b):
                t = {}
                t["xb"] = rpool.tile([P, 5, D], BF16, tag="resid", name=f"xb{b}")
                t["xn8"] = fmpool.tile([P, KK, SP], FP8, tag="xn8", name=f"xn8_{b}")
                t["q8"] = fmpool.tile([P, KK, SP], FP8, tag="q8", name=f"q8_{b}")
                t["k8"] = fmpool.tile([P, KK, SP], FP8, tag="k8", name=f"k8_{b}")
                t["v8"] = fmpool.tile([P, 5, H * VS], FP8, tag="v8", name=f"v8_{b}")
                t["ctx8"] = fmpool.tile([P, KK, SP], FP8, tag="ctx8", name=f"ctx8_{b}")
                return t

            # ---- layernorm helpers ----
            # stats cols: 0:5 negmu, 5:10 sumsq, 10:15 mu2, 15:20 var,
            # 20:25 lnv, 25:30 rsig
            def ln_new_stats():
                st = lnpool.tile([P, 35], F32, tag="stats", name="stats")
                nc.vector.memset(st[:, 0:5], 0.0)
                nc.vector.memset(st[:, 5:10], float(D))
                return st

            def ln_tile_stats(st, src, ti, pt, act_only=False):
                # sum-of-x on Act (act_only, keeps DVE clear at startup) or
                # DVE (tensor_reduce); sum-of-squares always on Act
                scr = lnpool.tile([P, D], BF16, tag="scr", bufs=1)
                if act_only:
                    nc.scalar.activation(scr[:pt], src[:pt, ti], AF.Identity,
                                         accum_out=st[:pt, ti:ti + 1])
                else:
                    nc.vector.tensor_reduce(st[:pt, ti:ti + 1], src[:pt, ti],
                                            mybir.AxisListType.X, OP.add)
                nc.scalar.activation(scr[:pt], src[:pt, ti], AF.Square,
                                     accum_out=st[:pt, 5 + ti:6 + ti])

            def ln_finalize(st, lo, hi):
                # rsig = rsqrt(var+eps) = exp(-0.5*ln(v)) with ln(v) by Taylor
                # around v=1 (v is ~1 +- 0.3 here), sharpened by one Newton
                # step.  Exp shares the softmax act table, so no table swaps.
                sumx = st[:, 0 + lo:0 + hi]
                negmu = st[:, 0 + lo:0 + hi]
                ssq = st[:, 5 + lo:5 + hi]
                t1 = st[:, 10 + lo:10 + hi]
                var = st[:, 15 + lo:15 + hi]
                u = st[:, 20 + lo:20 + hi]
                rsig = st[:, 25 + lo:25 + hi]
                p = st[:, 30 + lo:30 + hi]
                nc.vector.tensor_scalar_mul(negmu, sumx, -1.0 / D)
                nc.vector.tensor_tensor(t1, negmu, negmu, OP.mult)
                nc.vector.scalar_tensor_tensor(var, ssq, 1.0 / D, t1,
                                               OP.mult, OP.subtract)
                nc.vector.tensor_scalar(var, var, 1.0, EPS, OP.mult, OP.add)
                nc.vector.tensor_scalar(u, var, 1.0, -1.0, OP.mult, OP.add)
                nc.vector.tensor_scalar(p, u, 0.25, -0.5, OP.mult, OP.add)
                nc.vector.tensor_tensor(u, u, p, OP.mult)
                nc.scalar.activation(rsig, u, AF.Exp)
                nc.vector.tensor_tensor(t1, rsig, rsig, OP.mult)
                nc.vector.tensor_tensor(t1, t1, var, OP.mult)
                nc.vector.tensor_scalar(t1, t1, -0.5, 1.5, OP.mult, OP.add)
                nc.vector.tensor_tensor(rsig, rsig, t1, OP.mult)

            def ln_apply(st, src, ti, pt, outs):
                # z = (x - mu) * rsig  (bf16, token-major), then transpose to
                # feature-major and write fp8 via `outs(pgrp, kk0, nkk, t0, pt)`
                t0 = TT[ti][0]
                z = lnpool.tile([P, D], BF16, tag="z")
                nc.vector.tensor_scalar(z[:pt], src[:pt, ti],
                                        st[:pt, ti:ti + 1],
                                        st[:pt, 25 + ti:26 + ti],
                                        OP.add, OP.mult)
                for g in range(2):
                    pT = psC.tile([P, 512], BF16, tag="pC")
                    for j in range(4):
                        kk = g * 4 + j
                        nc.tensor.transpose(pT[:, j * P:j * P + pt],
                                            z[:pt, kk * P:(kk + 1) * P],
                                            ident[:pt, :pt])
                    pgrp = pT[:].rearrange("p (j c) -> p j c", j=4)[:, :, :pt]
                    outs(pgrp, g * 4, 4, t0, pt)

            # ==== stage emitters (generators yield emission quanta) ====

            def ln1_quanta(b, t):
                def q0():
                    nc.vector.memset(t["xb"][64:, 4, :], 0.0)
                    # tile 0 alone first (its LN chain is the startup
                    # critical path), then tiles 1-3 as one transfer
                    nc.sync.dma_start(t["xb"][:, 0, :], x_d[b, 0:P, :])
                    nc.sync.dma_start(
                        t["xb"][:, 1:4, :],
                        x_d[b, P:512].rearrange("(t p) d -> p t d", p=P))
                    nc.sync.dma_start(t["xb"][:65, 4], x_d[b, 512:S, :])
                    t["st1"] = ln_new_stats()
                yield q0

                def xn_out(pgrp, kk0, nkk, t0, pt):
                    nc.vector.tensor_copy(
                        t["xn8"][:, kk0:kk0 + nkk, t0:t0 + pt], pgrp)

                def fin0():
                    ln_tile_stats(t["st1"], t["xb"], 0, 128, act_only=(b == 0))
                    ln_finalize(t["st1"], 0, 1)
                    ln_apply(t["st1"], t["xb"], 0, 128, xn_out)
                yield fin0

                def fin1():
                    for ti in (1, 2, 3):
                        ln_tile_stats(t["st1"], t["xb"], ti, 128,
                                      act_only=(b == 0))
                    ln_finalize(t["st1"], 1, 4)
                yield fin1
                for ti in (1, 2, 3):
                    yield lambda ti=ti: ln_apply(t["st1"], t["xb"], ti, TT[ti][1], xn_out)

                def fin4():
                    ln_tile_stats(t["st1"], t["xb"], 4, 66, act_only=(b == 0))
                    ln_finalize(t["st1"], 4, 5)
                    ln_apply(t["st1"], t["xb"], 4, 66, xn_out)
                yield fin4

            def proj_quanta(b, t, wq_sb, wk_sb, wv_sb):
                # q/k per (m-tile, token-chunk); QC0 chunks first since they
                # only need xn8 token tiles 0-2 (tile 4 lands last at startup)
                def qk_m(m, qi):
                    q0, qn = QC[qi]
                    for w_sb, dst, bias in ((wq_sb, t["q8"], bq_sb),
                                            (wk_sb, t["k8"], bk_sb)):
                        ps = psA.tile([P, 512], F32, tag="pA")
                        for c in range(CP):
                            nc.tensor.matmul(
                                ps[:, :qn],
                                w_sb[:, c, :, m * P:(m + 1) * P],
                                t["xn8"][:, 2 * c:2 * c + 2, q0:q0 + qn],
                                start=(c == 0), stop=(c == CP - 1),
                                perf_mode=DR)
                        nc.vector.tensor_scalar(
                            dst[:, m, q0:q0 + qn], ps[:, :qn],
                            1.0 / SW, bias[:, m:m + 1], OP.mult, OP.add)
                for m in range(KK):
                    yield lambda m=m: qk_m(m, 0)
                for m in range(KK):
                    yield lambda m=m: qk_m(m, 1)
                # v: ones column / pad-row zeroing, then the projection
                def v_init():
                    v_hc = t["v8"][:].rearrange("p t (h c) -> p t h c", c=VS)
                    nc.vector.memset(v_hc[64:, 4:5], 0.0)
                    nc.vector.memset(v_hc[:, :, :, 65:VS], 0.0)
                    nc.vector.tensor_copy(
                        v_hc[:, 0:4, :, 64:65],
                        onec[:, :, None, None].to_broadcast((P, 4, H, 1)))
                    nc.vector.tensor_copy(
                        v_hc[:65, 4:5, :, 64:65],
                        onec[:65, :, None, None].to_broadcast((65, 1, H, 1)))
                yield v_init
                for ci, (c0, cn) in enumerate(DC):
                    for ti, (t0, pt) in enumerate(TT):
                        def v_t(ci=ci, c0=c0, cn=cn, ti=ti, t0=t0, pt=pt):
                            ps = psA.tile([P, 512], F32, tag="pA")
                            for c in range(CP):
                                nc.tensor.matmul(
                                    ps[:pt],
                                    t["xn8"][:, 2 * c:2 * c + 2, t0:t0 + pt],
                                    wv_sb[:, c, :, c0:c0 + cn],
                                    start=(c == 0), stop=(c == CP - 1),
                                    perf_mode=DR)
                            rp = min(pt, S - t0)
                            nc.vector.tensor_scalar_mul(
                                t["v8"][:rp, ti].rearrange(
                                    "p (h c) -> p h c", c=VS)[:, ci * 8:(ci + 1) * 8, 0:64],
                                ps[:rp, :cn].rearrange("p (h c) -> p h c", c=64),
                                1.0 / SW)
                        yield v_t

            def attn_quanta(b, t):
                # per (head, qc) chunk: scores + exp; pv/normalize for the
                # previous chunk trails one step so exp latency is hidden
                def scores_exp(h, qi):
                    hrow = (h % 2) * 64
                    kkh = h // 2
                    q0, qn = QC[qi]
                    es = spool.tile([P, 5, 290], FP8, tag=f"es{qi}", bufs=2)
                    t[f"es_{h}_{qi}"] = es
                    # token tiles 0-3 share one 4-bank psum group -> one Exp
                    pg = psS.tile([P, 4, 512], F32, tag="pS", bufs=1)
                    for kt in range(4):
                        kt0, ptk = TT[kt]
                        nc.tensor.matmul(
                            pg[:ptk, kt, :qn],
                            t["k8"][hrow:hrow + 64, kkh, kt0:kt0 + ptk],
                            t["q8"][hrow:hrow + 64, kkh, q0:q0 + qn],
                            start=True, stop=True)
                    nc.scalar.activation(
                        es[:, 0:4, :qn], pg[:, :, :qn],
                        AF.Exp, scale=1.0 / np.sqrt(DH), bias=expb[:])
                    pg4 = psA.tile([P, 512], F32, tag="pA")
                    kt0, ptk = TT[4]
                    nc.tensor.matmul(
                        pg4[:ptk, :qn],
                        t["k8"][hrow:hrow + 64, kkh, kt0:kt0 + ptk],
                        t["q8"][hrow:hrow + 64, kkh, q0:q0 + qn],
                        start=True, stop=True)
                    nc.scalar.activation(
                        es[:ptk, 4, :qn], pg4[:ptk, :qn],
                        AF.Exp, scale=1.0 / np.sqrt(DH), bias=expb[:ptk])

                def pv_norm(h, qi):
                    hrow = (h % 2) * 64
                    kkh = h // 2
                    q0, qn = QC[qi]
                    es = t.pop(f"es_{h}_{qi}")
                    pc = psC.tile([P, 512], F32, tag="pC")
                    for kt in (0, 2):
                        nc.tensor.matmul(
                            pc[:VS, :qn],
                            t["v8"][:, kt:kt + 2, h * VS:(h + 1) * VS],
                            es[:, kt:kt + 2, :qn],
                            start=(kt == 0), stop=False, perf_mode=DR)
                    nc.tensor.matmul(
                        pc[:VS, :qn], t["v8"][:66, 4, h * VS:(h + 1) * VS],
                        es[:66, 4, :qn], start=False, stop=True)
                    rc = spool.tile([1, 290], F32, tag="rc", bufs=2)
                    nc.vector.reciprocal(rc[:, :qn], pc[64:65, :qn])
                    rb = spool.tile([64, 290], F32, tag="rb", bufs=2)
                    nc.gpsimd.partition_broadcast(rb[:, :qn], rc[:, :qn])
                    # ctx stays on DVE (reads PSUM); move the copy of pc out
                    # of PSUM... (gpsimd is SBUF-only, so DVE does this one)
                    nc.vector.tensor_tensor(
                        t["ctx8"][hrow:hrow + 64, kkh, q0:q0 + qn],
                        pc[0:64, :qn], rb[:, :qn], OP.mult)

                prev = None
                for h in range(H):
                    for qi in range(2):
                        def quantum(h=h, qi=qi, prev=prev):
                            scores_exp(h, qi)
                            if prev is not None:
                                pv_norm(*prev)
                        yield quantum
                        prev = (h, qi)
                yield lambda: pv_norm(*prev)

            def stage_e_quanta(b, t, wstream):
                # x2 = ctx@wo/SW + bo + xb, with LN2 stats folded per tile;
                # wo streamed per 512-column half to save SBUF
                def q0():
                    t["x2"] = rpool.tile([P, 5, D], BF16, tag="resid",
                                         name=f"x2_{b}")
                    t["st2"] = ln_new_stats()
                yield q0
                for ci, (c0, cn) in enumerate(DC):
                    def ldwo(ci=ci, c0=c0, cn=cn):
                        wo_sb = wstream.tile([P, CP, 2, 512], FP8, tag="wo",
                                             bufs=1, name=f"wo{ci}_{b}")
                        t["wo_sb"] = wo_sb
                        nc.sync.dma_start(wo_sb[:], wo_d[:, :, :, c0:c0 + cn])
                    yield ldwo
                    for ti, (t0, pt) in enumerate(TT):
                        def eq(ci=ci, c0=c0, cn=cn, ti=ti, t0=t0, pt=pt):
                            wo_sb = t["wo_sb"]
                            ps = psA.tile([P, 512], F32, tag="pA")
                            nc.tensor.matmul(ps[:pt, :cn], ones8[:, :pt],
                                             t_bo8[:, c0:c0 + cn],
                                             start=True, stop=False)
                            for c in range(CP):
                                nc.tensor.matmul(
                                    ps[:pt, :cn],
                                    t["ctx8"][:, 2 * c:2 * c + 2, t0:t0 + pt],
                                    wo_sb[:, c, :, :cn],
                                    start=False, stop=(c == CP - 1),
                                    perf_mode=DR)
                            nc.vector.scalar_tensor_tensor(
                                t["x2"][:pt, ti, c0:c0 + cn], ps[:pt, :cn],
                                1.0 / SW, t["xb"][:pt, ti, c0:c0 + cn],
                                OP.mult, OP.add)
                            if ci == len(DC) - 1:
                                ln_tile_stats(t["st2"], t["x2"], ti, pt)
                        yield eq

            def ln2_quanta(b, t):
                def q0():
                    t["x2h"] = fmpool.tile([P, KK, SP], FP8, tag="x2h",
                                           bufs=1, name=f"x2h_{b}")
                    t["x2l"] = fmpool.tile([P, KK, SP], FP8, tag="x2l",
                                           bufs=1, name=f"x2l_{b}")
                    ln_finalize(t["st2"], 0, 1)
                yield q0

                def xn2_out(pgrp, kk0, nkk, t0, pt):
                    # hi-copy on Act (idle here), lo on DVE: halves the
                    # serial DVE chain that gates fc1
                    hi = t["x2h"][:, kk0:kk0 + nkk, t0:t0 + pt]
                    nc.scalar.copy(hi, pgrp)
                    nc.vector.scalar_tensor_tensor(
                        t["x2l"][:, kk0:kk0 + nkk, t0:t0 + pt],
                        pgrp, 1.0, hi, OP.mult, OP.subtract)

                for ti, (t0, pt) in enumerate(TT):
                    def ap(ti=ti, t0=t0, pt=pt):
                        if ti == 1:
                            ln_finalize(t["st2"], 1, 5)
                        ln_apply(t["st2"], t["x2"], ti, pt, xn2_out)
                    yield ap

            def fc1_quanta(b, t, wstream):
                # fc1: 3-term DR; gelu once to bf16 tmp, split hi/lo from it.
                # Kept in its own phase so Gelu never interleaves with Exp
                # (each act-table swap costs 1.3us on the Act engine).
                h1h = h1pool.tile([P, FK, SP], FP8, tag="h1h", name=f"h1h_{b}")
                t["h1h"] = h1h

                def ldw1(blk):
                    wb = wstream.tile([P, CP, 2, 2, 512], FP8, tag="w1",
                                      bufs=2, name=f"w1b{blk}")
                    t[f"w1_{blk}"] = wb
                    nc.sync.dma_start(
                        wb[:], w1_d[:, :, :, :, blk * 512:(blk + 1) * 512])

                for blk in range(8):          # 512-feature blocks
                    # prefetch distance 1: load blk+1 while blk computes
                    if blk == 0:
                        yield lambda: (ldw1(0), ldw1(1))
                    elif blk < 7:
                        yield lambda blk=blk: ldw1(blk + 1)
                    for mi in range(4):
                        def fc1_m(blk=blk, mi=mi):
                            wb = t[f"w1_{blk}"]
                            m = blk * 4 + mi
                            terms = ((0, t["x2h"]), (1, t["x2h"]), (0, t["x2l"]))
                            for (q0, qn) in QC:
                                ps = psA.tile([P, 512], F32, tag="pA")
                                n = CP * len(terms)
                                i = 0
                                for c in range(CP):
                                    for wj, xh in terms:
                                        nc.tensor.matmul(
                                            ps[:, :qn],
                                            wb[:, c, :, wj, mi * P:(mi + 1) * P],
                                            xh[:, 2 * c:2 * c + 2, q0:q0 + qn],
                                            start=(i == 0), stop=(i == n - 1),
                                            perf_mode=DR)
                                        i += 1
                                nc.scalar.activation(
                                    h1h[:, m, q0:q0 + qn], ps[:, :qn], _GELU,
                                    scale=1.0 / SW, bias=b1_sb[:, m:m + 1])
                        yield fc1_m

            def fc2_quanta(b, t, wstream):
                # fc2: 3-term DR over F; readout -> bf16 -> transpose -> +x2
                h1h = t["h1h"]
                for m in range(KK):
                    def ldw2(m=m):
                        wb = wstream.tile([P, CP2, 2, 2, P], FP8, tag="w2",
                                          bufs=2, name=f"w2b{m}")
                        t[f"w2_{m}"] = wb
                        nc.gpsimd.dma_start(wb[:], w2_d[m])
                    yield ldw2

                    def fc2_m(m=m):
                        wb = t.pop(f"w2_{m}")
                        mlp_m = spool.tile([P, SP], BF16, tag="mlpm", bufs=2)
                        terms = ((0, h1h), (1, h1h))
                        for (q0, qn) in QC:
                            ps = psA.tile([P, 512], F32, tag="pA")
                            n = CP2 * len(terms)
                            i = 0
                            for c in range(CP2):
                                for wj, xh in terms:
                                    nc.tensor.matmul(
                                        ps[:, :qn],
                                        wb[:, c, :, wj, :],
                                        xh[:, 2 * c:2 * c + 2, q0:q0 + qn],
                                        start=(i == 0), stop=(i == n - 1),
                                        perf_mode=DR)
                                    i += 1
                            nc.vector.tensor_scalar(
                                mlp_m[:, q0:q0 + qn], ps[:, :qn],
                                1.0 / SW2, b2_sb[:, m:m + 1], OP.mult, OP.add)
                        # transpose back token-major, add residual, stage for DMA
                        pT = psC.tile([P, 512], BF16, tag="pC")
                        for ti in range(4):
                            nc.tensor.transpose(
                                pT[:, ti * P:(ti + 1) * P],
                                mlp_m[:, TT[ti][0]:TT[ti][0] + 128],
                                ident[:])
                        pT2 = psC.tile([P, 512], BF16, tag="pC")
                        nc.tensor.transpose(pT2[:66, :P], mlp_m[:, 512:578],
                                            ident[:])
                        if m % 4 == 0:
                            t["og"] = [spool.tile([P, 4, P], F32, tag="og", bufs=5,
                                                  name=f"og{ti}")
                                       for ti in range(5)]
                        for ti, (t0, pt) in enumerate(TT):
                            src = (pT[:pt, ti * P:(ti + 1) * P] if ti < 4
                                   else pT2[:pt, 0:P])
                            nc.vector.scalar_tensor_tensor(
                                t["og"][ti][:pt, m % 4, :], src, 1.0,
                                t["x2"][:pt, ti, m * P:(m + 1) * P],
                                OP.mult, OP.add)
                        if m % 4 == 3:
                            mg = m // 4
                            for ti, (t0, pt) in enumerate(TT):
                                rp = min(pt, S - t0)
                                eng = nc.gpsimd if ti % 2 == 0 else nc.sync
                                eng.dma_start(
                                    y_d[b, t0:t0 + rp, mg * 512:(mg + 1) * 512],
                                    t["og"][ti][:rp])
                    yield fc2_m

            # ==== driver: software-pipeline the two batches ====

            def run_seq(gens):
                for g in gens:
                    for q in g:
                        q()

            def interleave(ga, gb, ratio=1):
                # ratio quanta of ga per quantum of gb
                ga, gb = iter(ga), iter(gb)
                alive_a = alive_b = True
                while alive_a or alive_b:
                    for _ in range(ratio):
                        if alive_a:
                            try:
                                next(ga)()
                            except StopIteration:
                                alive_a = False
                    if alive_b:
                        try:
                            next(gb)()
                        except StopIteration:
                            alive_b = False

            def chain(*gens):
                for g in gens:
                    yield from g

            tiles = [batch_tiles(0), batch_tiles(1)]

            with tc.tile_pool(name="wqkv", bufs=1) as qkvpool:
                wq_sb = qkvpool.tile([P, CP, 2, D], FP8, tag="wq")
                wk_sb = qkvpool.tile([P, CP, 2, D], FP8, tag="wk")
                wv_sb = qkvpool.tile([P, CP, 2, D], FP8, tag="wv")
                # x(0) first so its tiles win the serial DMA device, then
                # weights on the Pool queue (idle until attention)
                g_ln1 = iter(ln1_quanta(0, tiles[0]))
                next(g_ln1)()
                nc.gpsimd.dma_start(wq_sb[:], wq_d)
                nc.gpsimd.dma_start(wk_sb[:], wk_d)
                nc.gpsimd.dma_start(wv_sb[:], wv_d)
                load_small_params()

                # ph0: LN1(0) + proj(0)
                run_seq([g_ln1,
                         proj_quanta(0, tiles[0], wq_sb, wk_sb, wv_sb)])
                # ph1: attention(0) [Act-bound] vs LN1(1)+proj(1) [PE-bound]
                interleave(attn_quanta(0, tiles[0]),
                           chain(ln1_quanta(1, tiles[1]),
                                 proj_quanta(1, tiles[1], wq_sb, wk_sb, wv_sb)))

            def take(g, n):
                for _ in range(n):
                    try:
                        yield next(g)
                    except StopIteration:
                        return

            with tc.tile_pool(name="wstr", bufs=1) as wstream:
                ag1 = iter(attn_quanta(1, tiles[1]))
                # ph2: stage E(0)+LN2(0) [serial-chain heavy] filled with the
                # first attention(1) chunks (same act table as LN)
                interleave(chain(stage_e_quanta(0, tiles[0], wstream),
                                 ln2_quanta(0, tiles[0])),
                           take(ag1, 12))
                # ph2.5: fc1(0) alone (PE-bound; keeps Gelu table resident)
                run_seq([fc1_quanta(0, tiles[0], wstream)])
                # ph3: fc2(0) [PE-bound, no Act] vs attention(1) [Act-bound];
                # hold back the last 2 fc2 m-tiles to fill ph4's chains
                fg0 = iter(fc2_quanta(0, tiles[0], wstream))
                interleave(ag1, take(fg0, 12), ratio=2)
                # ph4: stage E(1) + LN2(1), filled with the fc2(0) tail
                fg1 = iter(fc1_quanta(1, tiles[1], wstream))
                next(fg1)()      # prefetch w1(1) blocks 0-1 during ph4
                interleave(chain(stage_e_quanta(1, tiles[1], wstream),
                                 ln2_quanta(1, tiles[1])),
                           fg0, ratio=4)
                # ph4.5 + ph5: MLP(1); first w1 blocks prefetched in ph4
                run_seq([fg1, fc2_quanta(1, tiles[1], wstream)])

    nc.compile()
    return nc


def _get_nc():
    global _NC_CACHE
    if _NC_CACHE is None:
        _NC_CACHE = _build()
    return _NC_CACHE


def _prep_inputs(inputs):
    """Host-side weight packing: LN-gain folds, bias folds, fp8 hi/lo splits."""
    f = {k: np.asarray(v, np.float32) for k, v in inputs.items()}
    g1, b1g = f["ln1_g"], f["ln1_b"]
    g2, b2g = f["ln2_g"], f["ln2_b"]

    def pack_d(w, ncp=CP):
        # [K, M] -> [P, ncp, 2, M] with K = ncp*2*P
        return np.ascontiguousarray(
            w.reshape(ncp, 2, P, -1).transpose(2, 0, 1, 3))

    def q8(a):
        return a.astype(E4NP)

    def split(a):
        hi = a.astype(E4NP)
        lo = (a - hi.astype(np.float32)).astype(E4NP)
        return hi, lo

    out = {}
    out["wq8"] = q8(pack_d(g1[:, None] * f["wq"] * SW))
    out["wk8"] = q8(pack_d(g1[:, None] * f["wk"] * SW))
    out["wv8"] = q8(pack_d(g1[:, None] * f["wv"] * SW))
    out["wo8"] = q8(pack_d(f["wo"] * SW))
    w1h, w1l = split(pack_d(g2[:, None] * f["w1"] * SW))
    out["w1p"] = np.ascontiguousarray(np.stack([w1h, w1l], axis=3))
    w2s = f["w2"] * SW2
    w2m = np.stack([pack_d(w2s[:, m * P:(m + 1) * P], CP2) for m in range(KK)])
    w2h, w2l = split(w2m)
    out["w2p"] = np.ascontiguousarray(np.stack([w2h, w2l], axis=4))

    bq_eff = b1g @ f["wq"] + f["bq"]
    bk_eff = b1g @ f["wk"] + f["bk"]
    bv_eff = b1g @ f["wv"] + f["bv"]
    out["bo8"] = ((bv_eff @ f["wo"] + f["bo"]) * SW).astype(E4NP)
    b1_eff = b2g @ f["w1"] + f["b1"]
    cap = np.zeros((P, 3 * KK + FK), np.float32)
    cap[:, 0:KK] = bq_eff.reshape(KK, P).T
    cap[:, KK:2 * KK] = bk_eff.reshape(KK, P).T
    cap[:, 2 * KK:3 * KK] = f["b2"].reshape(KK, P).T
    cap[:, 3 * KK:] = b1_eff.reshape(FK, P).T
    out["cap"] = cap
    return out


def kernel(**inputs):
    nc = _get_nc()
    x = np.asarray(inputs["x"], np.float32).astype(BFNP)
    shared = _prep_inputs(inputs)
    in_maps = []
    for i in range(NCORES):
        m = dict(shared)
        m["x"] = np.ascontiguousarray(x[i * BL:(i + 1) * BL])
        in_maps.append(m)
    res = bass_utils.run_bass_kernel_spmd(nc, in_maps, core_ids=list(range(NCORES)))
    y = np.concatenate([res.results[i]["y"] for i in range(NCORES)], axis=0)
    return y.astype(np.float32)
